# revision 21
# baseline (speedup 1.0000x reference)
"""Causal self-attention (B=4, N=2048, D=1024, H=16, hd=64) on 8 trn2 cores.

Sharding: core c -> (batch b = c//2, head-group hg = c%2 of 8 heads).
All matmul operands bf16 (PSUM accumulation f32). Per core:
  K^T,Q^T = (x @ Wkq + b)^T via W-stationary matmuls (Q pre-scaled 1/8),
    rows packed [K_h1|K_h2] / [Q_h1|Q_h2] per head-pair.
  V computed directly in [k, e] layout via x-stationary matmuls (lhsT = xT
    n-block, moving = Wv for all 8 heads); no transposes. V bias is added on
    the host after normalization (attn rows sum to 1).
  S^T[k, q] = K^T.T @ Q^T (row-packed pair of K=64 matmuls, causal blocks),
    P^T = exp(S^T) via ACT psum->sbuf bf16, triangular mask on diag blocks
    (one broadcast DVE multiply covering both heads).
  numerator^T (+ denominator row) = [V|1]-stationary matmuls over k blocks;
    shipped to host unnormalized; host divides, adds V bias, transposes.
"""

import sys

sys.path.insert(0, "/opt/trn_rl_repo")

import numpy as np
import ml_dtypes

BF16NP = ml_dtypes.bfloat16

B, N, D, H, HD = 4, 2048, 1024, 16, 64
NPAIR, DB, QC = 4, 8, 4  # head-pairs per core, 128-d-blocks, 512-q-chunks
NB = N // 128  # 16 k-blocks
SCALE = 1.0 / np.sqrt(HD)

_PROG_CACHE = {}


def build_program(rep=1):
    from concourse import bacc
    import concourse.bass as bass
    import concourse.mybir as mybir
    from concourse.tile import TileContext

    F32, BF = mybir.dt.float32, mybir.dt.bfloat16

    nc = bacc.Bacc("TRN2", target_bir_lowering=False)
    xt_d = nc.declare_dram_parameter("xt", [128, DB, N], BF, isOutput=False)
    wkq_d = nc.declare_dram_parameter(
        "wkq", [NPAIR, 128, DB, 256], BF, isOutput=False
    )
    wv_d = nc.declare_dram_parameter("wv", [128, DB, 512], BF, isOutput=False)
    bias_d = nc.declare_dram_parameter("bias", [128, NPAIR, 2], F32, isOutput=False)
    mask_d = nc.declare_dram_parameter("mask", [128, 128], BF, isOutput=False)
    out_d = nc.declare_dram_parameter(
        "outt", [NPAIR, 2, 65, N], F32, isOutput=True
    )

    with TileContext(nc) as tc:
        with (
            tc.tile_pool(name="big", bufs=1) as big,
            tc.tile_pool(name="wp", bufs=2) as wp,
            tc.tile_pool(name="projp", bufs=2) as projp,
            tc.tile_pool(name="attp", bufs=3) as attp,
            tc.tile_pool(name="psA", bufs=2, space="PSUM") as psA,
        ):
            mask = big.tile([128, 128], BF)
            nc.sync.dma_start(out=mask, in_=mask_d[:, :])
            biasb = big.tile([128, NPAIR, 2], F32)
            nc.sync.dma_start(out=biasb, in_=bias_d[:, :, :])

            w_tiles = {}

            def get_w(r, p):
                # Hoistable so pair-0 weights land before the bulk x/Wv loads.
                if (r, p) not in w_tiles:
                    w = wp.tile([128, DB, 256], BF, tag="w")
                    nc.sync.dma_start(out=w, in_=wkq_d[p])
                    w_tiles[(r, p)] = w
                return w_tiles[(r, p)]

            get_w(0, 0)
            xt = big.tile([128, DB, N], BF)
            for n4 in range(QC):
                nc.sync.dma_start(
                    out=xt[:, :, n4 * 512 : (n4 + 1) * 512],
                    in_=xt_d[:, :, n4 * 512 : (n4 + 1) * 512],
                )
            wv = big.tile([128, DB, 512], BF)
            nc.sync.dma_start(out=wv, in_=wv_d[:, :, :])
            vall = big.tile([128, NB, 8, 65], BF)
            # ones column via exp(0): also pulls the ACT exp table load into
            # the startup DMA window instead of the first attention block.
            zsrc = big.tile([128, 128], F32)
            nc.vector.memset(zsrc, 0.0)
            nc.scalar.activation(
                vall[:, :, :, 64:65],
                zsrc[:, :].rearrange("p (a b o) -> p a b o", a=NB, b=8, o=1),
                mybir.ActivationFunctionType.Exp,
            )

            for r in range(rep):
                for p in range(NPAIR):
                    w = get_w(r, p)

                    # ---- K^T, Q^T projection (W-stationary)
                    kt = projp.tile([128, N], BF, tag="kt")
                    qt = projp.tile([128, N], BF, tag="qt")
                    dests = [kt, qt]
                    for m in range(2):
                        for n4 in range(QC):
                            pp = psA.tile([128, 512], F32, tag="proj")
                            for db in range(DB):
                                nc.tensor.matmul(
                                    pp,
                                    w[:, db, m * 128 : (m + 1) * 128],
                                    xt[:, db, n4 * 512 : (n4 + 1) * 512],
                                    start=(db == 0),
                                    stop=(db == DB - 1),
                                )
                            nc.vector.tensor_scalar_add(
                                dests[m][:, n4 * 512 : (n4 + 1) * 512],
                                pp,
                                biasb[:, p, m : m + 1],
                            )

                    # ---- attention, per 512-wide q chunk
                    osbs = [
                        attp.tile(
                            [65, N], F32, tag=f"osb{h2}", name=f"osb{h2}", bufs=2
                        )
                        for h2 in range(2)
                    ]
                    for qc in range(QC):
                        if p == 0:
                            # V-direct (x-stationary) for the k-blocks this
                            # q-chunk unlocks; all 8 heads at once.
                            for nb in range(4 * qc, 4 * qc + 4):
                                pv = psA.tile([128, 512], F32, tag="proj")
                                for db in range(DB):
                                    nc.tensor.matmul(
                                        pv,
                                        xt[:, db, nb * 128 : (nb + 1) * 128],
                                        wv[:, db, :],
                                        start=(db == 0),
                                        stop=(db == DB - 1),
                                    )
                                nc.vector.tensor_copy(
                                    vall[:, nb, :, 0:64],
                                    pv[:, :].rearrange("p (h e) -> p h e", h=8),
                                )
                        po = [
                            psA.tile([128, 512], F32, tag="o", name=f"po{h2}")
                            for h2 in range(2)
                        ]
                        nkb = 4 * qc + 4
                        for kb in range(nkb):
                            q0 = 128 * max(0, kb - 4 * qc)
                            ps_s = psA.tile([128, 1024], F32, tag="s")
                            for h2 in range(2):
                                nc.tensor.matmul(
                                    ps_s[:, h2 * 512 + q0 : (h2 + 1) * 512],
                                    kt[
                                        h2 * 64 : (h2 + 1) * 64,
                                        kb * 128 : (kb + 1) * 128,
                                    ],
                                    qt[
                                        h2 * 64 : (h2 + 1) * 64,
                                        qc * 512 + q0 : (qc + 1) * 512,
                                    ],
                                    start=True,
                                    stop=True,
                                    tile_position=(h2 * 64, 0),
                                )
                            pt = attp.tile([128, 1024], BF, tag="pt", bufs=6)
                            sv = ps_s[:, :].rearrange("p (b w) -> p b w", b=2)
                            tv = pt[:, :].rearrange("p (b w) -> p b w", b=2)
                            nc.scalar.activation(
                                tv[:, :, q0:512],
                                sv[:, :, q0:512],
                                mybir.ActivationFunctionType.Exp,
                            )
                            if kb >= 4 * qc:  # diagonal: triangular mask
                                dv = tv[:, :, q0 : q0 + 128]
                                nc.vector.tensor_mul(
                                    dv,
                                    dv,
                                    mask[:, :]
                                    .rearrange("p (o w) -> p o w", o=1)
                                    .broadcast_to([128, 2, 128]),
                                )
                            for h2 in range(2):
                                nc.tensor.matmul(
                                    po[h2][0:65, q0:512],
                                    vall[:, kb, 2 * p + h2, :],
                                    pt[:, h2 * 512 + q0 : (h2 + 1) * 512],
                                    start=(kb == 0),
                                    stop=(kb == nkb - 1),
                                )
                        # drain: numerator + denominator row to SBUF
                        for h2 in range(2):
                            nc.vector.tensor_copy(
                                osbs[h2][:, qc * 512 : (qc + 1) * 512],
                                po[h2][0:65, :],
                            )
                    for h2 in range(2):
                        nc.sync.dma_start(out=out_d[p, h2], in_=osbs[h2])

    nc.compile()
    return nc


def get_program(rep=1):
    if rep not in _PROG_CACHE:
        _PROG_CACHE[rep] = build_program(rep)
    return _PROG_CACHE[rep]


def prep_inputs(x, W, b):
    x = np.asarray(x, dtype=np.float32)
    W = np.asarray(W, dtype=np.float32)
    b = np.asarray(b, dtype=np.float32)
    mask = (np.arange(128)[:, None] <= np.arange(128)[None, :]).astype(BF16NP)

    in_maps = []
    for c in range(8):
        bc, hg = divmod(c, 2)
        xt = np.ascontiguousarray(
            x[bc].T.reshape(DB, 128, N).transpose(1, 0, 2)
        ).astype(BF16NP)  # [128(dlow), DB, N]
        wkq = np.empty((NPAIR, 128, DB, 256), np.float32)
        bias = np.empty((128, NPAIR, 2), np.float32)
        for p in range(NPAIR):
            g1, g2 = hg * 8 + 2 * p, hg * 8 + 2 * p + 1
            Wp = np.empty((D, 256), np.float32)
            Wp[:, 0:64] = W[g1, :, 0:64]
            Wp[:, 64:128] = W[g2, :, 0:64]
            Wp[:, 128:192] = W[g1, :, 64:128] * SCALE
            Wp[:, 192:256] = W[g2, :, 64:128] * SCALE
            wkq[p] = Wp.reshape(DB, 128, 256).transpose(1, 0, 2)
            bias[0:64, p, 0] = b[g1, 0:64]
            bias[64:128, p, 0] = b[g2, 0:64]
            bias[0:64, p, 1] = b[g1, 64:128] * SCALE
            bias[64:128, p, 1] = b[g2, 64:128] * SCALE
        # V weights for all 8 local heads, natural [D, e] layout
        Wv = np.concatenate(
            [W[hg * 8 + h, :, 128:192] for h in range(8)], axis=1
        )  # [D, 512]
        wv = Wv.reshape(DB, 128, 512).transpose(1, 0, 2).astype(BF16NP)
        in_maps.append(
            {
                "xt": xt,
                "wkq": wkq.astype(BF16NP),
                "wv": wv,
                "bias": bias,
                "mask": mask,
            }
        )
    return in_maps


def run(nc, in_maps):
    from concourse.bass_utils import run_bass_kernel_spmd

    return run_bass_kernel_spmd(nc, in_maps, list(range(8)))


class Runner:
    """Persistent PJRT executable for an nc program: loads the NEFF once and
    reuses it across calls (run_bass_via_pjrt reloads per call)."""

    def __init__(self, nc, n_cores=8):
        import jax
        import numpy as np
        from jax.sharding import Mesh, PartitionSpec
        from jax.experimental.shard_map import shard_map
        import concourse.mybir as mybir
        from concourse import bass2jax

        bass2jax.install_neuronx_cc_hook()
        self.n_cores = n_cores
        partition_name = (
            nc.partition_id_tensor.name if nc.partition_id_tensor else None
        )
        in_names, out_names, out_avals, zero_outs = [], [], [], []
        for alloc in nc.m.functions[0].allocations:
            if not isinstance(alloc, mybir.MemoryLocationSet):
                continue
            name = alloc.memorylocations[0].name
            if alloc.kind == "ExternalInput":
                if name != partition_name:
                    in_names.append(name)
            elif alloc.kind == "ExternalOutput":
                shape = tuple(alloc.tensor_shape)
                dtype = mybir.dt.np(alloc.dtype)
                out_names.append(name)
                out_avals.append(jax.core.ShapedArray(shape, dtype))
                zero_outs.append(np.zeros(shape, dtype))
        n_params = len(in_names)
        all_in_names = list(in_names) + list(out_names)
        if partition_name is not None:
            all_in_names.append(partition_name)

        def _body(*args):
            operands = list(args)
            if partition_name is not None:
                operands.append(bass2jax.partition_id_tensor())
            outs = bass2jax._bass_exec_p.bind(
                *operands,
                out_avals=tuple(out_avals),
                in_names=tuple(all_in_names),
                out_names=tuple(out_names),
                lowering_input_output_aliases=(),
                sim_require_finite=True,
                sim_require_nnan=True,
                nc=nc,
            )
            return tuple(outs)

        devices = jax.devices()[:n_cores]
        mesh = Mesh(np.asarray(devices), ("core",))
        in_specs = (PartitionSpec("core"),) * (n_params + len(out_names))
        out_specs = (PartitionSpec("core"),) * len(out_names)
        self._fn = jax.jit(
            shard_map(
                _body,
                mesh=mesh,
                in_specs=in_specs,
                out_specs=out_specs,
                check_rep=False,
            ),
            keep_unused=True,
        )
        self.in_names, self.out_names = in_names, out_names
        self.out_avals, self.zero_outs = out_avals, zero_outs
        self.n_params = n_params
        self._jax = jax

    def put_inputs(self, in_maps):
        import numpy as np

        concat_in = [
            np.concatenate([np.asarray(m[n]) for m in in_maps], axis=0)
            for n in self.in_names
        ]
        concat_zeros = [
            np.zeros((self.n_cores * z.shape[0], *z.shape[1:]), z.dtype)
            for z in self.zero_outs
        ]
        return [self._jax.device_put(a) for a in concat_in + concat_zeros]

    def execute(self, dev_args):
        outs = self._fn(*dev_args)
        self._jax.block_until_ready(outs)
        return outs

    def run(self, in_maps):
        import numpy as np

        outs = self.execute(self.put_inputs(in_maps))
        return [
            {
                n: np.asarray(outs[i]).reshape(
                    self.n_cores, *self.out_avals[i].shape
                )[c]
                for i, n in enumerate(self.out_names)
            }
            for c in range(self.n_cores)
        ]


def assemble(results, b):
    b = np.asarray(b, dtype=np.float32)
    out = np.empty((B, N, D), np.float32)
    for c in range(8):
        bc, hg = divmod(c, 2)
        o = results[c]["outt"]  # [NPAIR, 2, 65, N]
        for p in range(NPAIR):
            for h2 in range(2):
                hglob = hg * 8 + 2 * p + h2
                num = o[p, h2, 0:64, :]  # [64, N]
                den = o[p, h2, 64:65, :]  # [1, N]
                out[bc, :, hglob * 64 : (hglob + 1) * 64] = (
                    (num / den).T + b[hglob, 128:192][None, :]
                )
    return out


def kernel(x, W, b):
    nc = get_program(rep=1)
    res = run(nc, prep_inputs(x, W, b))
    return assemble(res.results, b)
